# revision 40
# baseline (speedup 1.0000x reference)
"""BBoxHead kernel for 8 Trainium2 NeuronCores.

Reference computation (per roi):
  x1 = relu(bn1(pooled_rois . conv1_w + b1))      # full 7x7x256 contraction -> 1024
  x2 = relu(bn2(x1 @ conv2_w + b2))               # 1024 -> 1024
  logits = x2 @ logits_w + logits_b               # 1024 -> 81
  probs  = softmax(logits)
  deltas = x2 @ delta_w + delta_b                 # 1024 -> 324 -> [81, 4]

Activations are kept feature-major on-chip (X^T layout, [features, rois]) so
every matmul consumes operands K-on-partitions with zero on-device transposes
of activations; the host pre-transposes the pooled rois once.  BN is folded
into a per-feature affine on the host and applied fused with ReLU in a single
ScalarE activation per tile (PSUM -> SBUF).

Two distribution strategies (BBOX_IMPL):
  dp      - data-parallel over rois (250/core); every core streams the full
            conv1_w.  Default.
  ksplit  - conv1's contraction split across cores + on-chip ReduceScatter.
            Kept for reference: the collective costs ~70-120us in this
            runtime, so it loses to dp despite moving half the bytes.

Matmul dtype (BBOX_MM_DTYPE) and measured results (8 cores, HW exec time of
the traced core; scale-relative absmax vs the fp32 jax reference):
  bf16 (default)  ~148-155us  err 4.9e-3   conv1 stream 25.7MB/core, HBM-bound
  f32r            ~260us      err 2.8e-4   TF32-like; weight loads serialize
  f32             ~431us      err ~1e-6    full fp32 (4 PE cycles/row)
"""

import os
import sys

sys.path.insert(0, "/opt/trn_rl_repo")
import numpy as np

N_ROIS = 2000
K1 = 12544          # 7*7*256 contraction for conv1
HID = 1024
NCLS = 81
NCLS_P = 128        # logits head padded to a full PE tile
NDEL = 324
NDEL_P = 384        # delta head padded to 3 full PE tiles
P = 128
KT1 = K1 // P       # 98 contraction tiles for conv1 (dp mode)
FT = HID // P       # 8 feature tiles
NCORES = 8
RPC = N_ROIS // NCORES  # 250 rois per core (dp mode)
NR = 256            # padded rois per core (f32r needs free dim >= 256)
BN_EPS = 1e-3

# Matmul operand dtype: "bf16" (1 cyc/row + fast weight load + half DMA),
# "f32r" (TF32-like, 1 cyc/row but serialized weight loads), "f32" (4 cyc/row).
MM_DTYPE = os.environ.get("BBOX_MM_DTYPE", "bf16")
USE_F32R = MM_DTYPE == "f32r"
IMPL = os.environ.get("BBOX_IMPL", "dp")

K1P = 13312          # conv1 contraction padded to 8*13 tiles of 128
KTC = 13             # conv1 k-tiles per core in ksplit mode
NRT = 2048           # padded total rois in ksplit mode (8 x 256)

_CACHE: dict = {}


def _mm_dt(mybir):
    return {"bf16": mybir.dt.bfloat16, "f32r": mybir.dt.float32r,
            "f32": mybir.dt.float32}[MM_DTYPE]


def _mk_io(nc, mybir, a_shape, w1_shape):
    f32 = mybir.dt.float32
    mm_dt = _mm_dt(mybir)
    io = {}
    io["a_t"] = nc.dram_tensor("a_t", a_shape, mm_dt, kind="ExternalInput")
    io["w1"] = nc.dram_tensor("w1", w1_shape, mm_dt, kind="ExternalInput")
    io["w2"] = nc.dram_tensor("w2", [HID, HID], mm_dt, kind="ExternalInput")
    io["w3"] = nc.dram_tensor("w3", [HID, NCLS_P], mm_dt, kind="ExternalInput")
    io["w4"] = nc.dram_tensor("w4", [HID, NDEL_P], mm_dt, kind="ExternalInput")
    for name, n in [("s1", HID), ("t1", HID), ("s2", HID), ("t2", HID),
                    ("b3", NCLS_P), ("b4", NDEL_P)]:
        io[name] = nc.dram_tensor(name, [n], f32, kind="ExternalInput")
    io["logits_out"] = nc.dram_tensor("logits_out", [NR, NCLS], f32, kind="ExternalOutput")
    io["probs_out"] = nc.dram_tensor("probs_out", [NR, NCLS], f32, kind="ExternalOutput")
    io["deltas_out"] = nc.dram_tensor("deltas_out", [NR, NDEL], f32, kind="ExternalOutput")
    return io


def _emit_tail(nc, mybir, tc, pools, io, x1_sb, w2_tiles, mm_dt):
    """conv2 + heads + softmax + transposed outputs, from feature-major x1."""
    from concourse.masks import make_identity

    f32 = mybir.dt.float32
    AF = mybir.ActivationFunctionType
    AX = mybir.AxisListType
    singles, psum, small = pools["singles"], pools["psum"], pools["small"]

    ident = singles.tile([P, P], f32)
    make_identity(nc, ident)

    s2_sb, t2_sb = pools["s2_sb"], pools["t2_sb"]
    b3_sb, b4_sb = pools["b3_sb"], pools["b4_sb"]
    w3_sb, w4_sb = pools["w3_sb"], pools["w4_sb"]

    # conv2: X2^T = W2^T @ X1^T
    c2 = psum.tile([P, FT, 512], f32, tag="acc")
    for f in range(FT):
        for k in range(FT):
            nc.tensor.matmul(
                c2[:, f, :NR],
                lhsT=w2_tiles(k, f),
                rhs=x1_sb[:, k, :],
                start=(k == 0),
                stop=(k == FT - 1),
            )

    x2_sb = pools["x2_sb"]
    for f in range(FT):
        if f % 2 == 0:
            nc.scalar.activation(
                out=x2_sb[:, f, :], in_=c2[:, f, :NR], func=AF.Relu,
                bias=t2_sb[:, f:f + 1], scale=s2_sb[:, f:f + 1],
            )
        else:
            nc.vector.tensor_scalar(
                x2_sb[:, f, :], c2[:, f, :NR], s2_sb[:, f:f + 1],
                t2_sb[:, f:f + 1], mybir.AluOpType.mult, mybir.AluOpType.add)
            nc.vector.tensor_scalar_max(x2_sb[:, f, :], x2_sb[:, f, :], 0.0)

    # heads: logits^T into bank 0, deltas^T into banks 1..3
    c3 = psum.tile([P, FT, 512], f32, tag="acc")
    for k in range(FT):
        nc.tensor.matmul(
            c3[:, 0, :NR], lhsT=w3_sb[:, k, :], rhs=x2_sb[:, k, :],
            start=(k == 0), stop=(k == FT - 1),
        )
    for m in range(3):
        for k in range(FT):
            nc.tensor.matmul(
                c3[:, 1 + m, :NR],
                lhsT=w4_sb[:, k, m * P:(m + 1) * P],
                rhs=x2_sb[:, k, :],
                start=(k == 0), stop=(k == FT - 1),
            )

    # head bias adds on DVE so they overlap ScalarE's BN2 work
    l_sb = small.tile([P, NR], f32, tag="l")
    nc.vector.tensor_scalar_add(l_sb, c3[:, 0, :NR], b3_sb[:, 0:1])
    d_sb = small.tile([P, 3, NR], f32, tag="d")
    for m in range(3):
        nc.vector.tensor_scalar_add(d_sb[:, m, :], c3[:, 1 + m, :NR],
                                    b4_sb[:, m:m + 1])

    # transpose heads back to roi-major: 2 logit blocks + 6 delta blocks
    c4 = psum.tile([P, FT, 512], f32, tag="acc")
    for j in range(2):
        nc.tensor.transpose(c4[:, j, :P], l_sb[:, j * P:(j + 1) * P], ident)
    for m in range(3):
        for j in range(2):
            nc.tensor.transpose(c4[:, 2 + m * 2 + j, :P],
                                d_sb[:, m, j * P:(j + 1) * P], ident)

    lg_sb = small.tile([P, 2, NCLS], f32, tag="lg")
    pr_sb = small.tile([P, 2, NCLS], f32, tag="pr")
    dl_sb = small.tile([P, 2, NDEL], f32, tag="dl")
    for j in range(2):
        nc.vector.tensor_copy(lg_sb[:, j, :], c4[:, j, :NCLS])
        negmax = small.tile([P, 1], f32, tag="nm")
        nc.vector.reduce_max(negmax, c4[:, j, :NCLS], axis=AX.X, negate=True)
        esum = small.tile([P, 1], f32, tag="es")
        nc.scalar.activation(out=pr_sb[:, j, :], in_=c4[:, j, :NCLS],
                             func=AF.Exp, bias=negmax, scale=1.0,
                             accum_out=esum)
        rsum = small.tile([P, 1], f32, tag="rs")
        nc.vector.reciprocal(rsum, esum)
        nc.vector.tensor_scalar_mul(pr_sb[:, j, :], pr_sb[:, j, :], rsum)
        for m in range(3):
            mw = NDEL - m * P if m == 2 else P
            nc.vector.tensor_copy(dl_sb[:, j, m * P:m * P + mw],
                                  c4[:, 2 + m * 2 + j, :mw])

    nc.sync.dma_start(io["logits_out"].ap().rearrange("(j p) c -> p j c", p=P), lg_sb)
    nc.sync.dma_start(io["probs_out"].ap().rearrange("(j p) c -> p j c", p=P), pr_sb)
    nc.sync.dma_start(io["deltas_out"].ap().rearrange("(j p) c -> p j c", p=P), dl_sb)


def _mk_vec_tiles(nc, mybir, singles, io):
    f32 = mybir.dt.float32

    def vec_tile(name, cols):
        t = singles.tile([P, cols], f32, tag=name + "_sb")
        nc.sync.dma_start(t, io[name].ap().rearrange("(o p) -> p o", p=P))
        return t

    return {
        "s1_sb": vec_tile("s1", FT), "t1_sb": vec_tile("t1", FT),
        "s2_sb": vec_tile("s2", FT), "t2_sb": vec_tile("t2", FT),
        "b3_sb": vec_tile("b3", 1), "b4_sb": vec_tile("b4", 3),
    }


def _build_program_dp():
    """Data-parallel: 250 rois/core, full conv1_w streamed on every core."""
    from concourse import bacc
    import concourse.mybir as mybir
    import concourse.tile as tile

    f32 = mybir.dt.float32
    mm_dt = _mm_dt(mybir)
    AF = mybir.ActivationFunctionType

    nc = bacc.Bacc("TRN2", target_bir_lowering=False, debug=False,
                   num_devices=NCORES)
    io = _mk_io(nc, mybir, [K1, NR], [K1, HID])

    AG, KPG = 14, 7  # stream a_t in 14 groups of 7 k-tiles

    with tile.TileContext(nc) as tc:
        with (
            tc.tile_pool(name="singles", bufs=1) as singles,
            tc.tile_pool(name="astream", bufs=9 if MM_DTYPE == "bf16" else 4) as apool,
            tc.tile_pool(name="wstream", bufs=8 if MM_DTYPE == "bf16" else 3) as wpool,
            tc.tile_pool(name="psum", bufs=1, space="PSUM") as psum,
            tc.tile_pool(name="small", bufs=2) as small,
        ):
            pools = {"singles": singles, "psum": psum, "small": small}

            # conv1: accumulate X1^T = W1^T @ A^T over 98 k-tiles.
            # w1 streams on the sync HWDGE queue; the a-groups ride gpsimd so
            # the two streams don't head-of-line block each other, and all
            # tail-only loads are emitted after the loop.
            a_t3 = io["a_t"].ap().rearrange("(kt p) n -> p kt n", p=P)
            c1 = psum.tile([P, FT, 512], f32, tag="acc")

            def schedule(sizes, total):
                out, k = [], 0
                for s in sizes:
                    out.append((k, s))
                    k += s
                while k < total:
                    s = min(sizes[-1], total - k)
                    out.append((k, s))
                    k += s
                assert k == total
                return out

            AGS = schedule([4, 7], KT1)             # a-groups
            WGS = schedule([2, 2, 4], KT1)          # w1-groups
            a_map = {}
            for st, sz in AGS:
                a_map[st] = (st, sz)
            w_map = {}
            for st, sz in WGS:
                w_map[st] = (st, sz)

            w2_sb = w3_sb = w4_sb = None
            a_g = w1_g = None
            a_st = w_st = 0
            wq = 0
            for k in range(KT1):
                if k in a_map:
                    st, sz = a_map[k]
                    a_g = apool.tile([P, 7, NR], mm_dt, tag="a", name="a_g")
                    a_st = st
                    nc.scalar.dma_start(a_g[:, :sz, :], a_t3[:, st:st + sz, :])
                if k in w_map:
                    st, sz = w_map[k]
                    w1_g = wpool.tile([P, 4, HID], mm_dt, tag="w1", name="w1_g")
                    w_st = st
                    nc.sync.dma_start(
                        w1_g[:, :sz, :],
                        io["w1"].ap().rearrange("(kt p) f -> p kt f", p=P)[:, st:st + sz, :])
                if k == 76:
                    # tail weights are queued on the in-order HWDGE queues
                    # BEHIND the k=76 w1 groups: queue order defers them past
                    # the bandwidth-critical conv1 head (an idle gpsimd queue
                    # would transfer them immediately and steal early HBM bw)
                    w2_sb = singles.tile([P, FT, HID], mm_dt, name="w2_sb")
                    w2_3 = io["w2"].ap().rearrange("(kt p) f -> p kt f", p=P)
                    nc.scalar.dma_start(w2_sb[:, 0:4, :], w2_3[:, 0:4, :])
                    nc.scalar.dma_start(w2_sb[:, 4:8, :], w2_3[:, 4:8, :])
                if k == 84:
                    w3_sb = singles.tile([P, FT, NCLS_P], mm_dt, name="w3_sb")
                    nc.scalar.dma_start(w3_sb, io["w3"].ap().rearrange("(kt p) f -> p kt f", p=P))
                    w4_sb = singles.tile([P, FT, NDEL_P], mm_dt, name="w4_sb")
                    nc.scalar.dma_start(w4_sb, io["w4"].ap().rearrange("(kt p) f -> p kt f", p=P))
                rhs = a_g[:, k - a_st, :]
                for f in range(FT):
                    nc.tensor.matmul(
                        c1[:, f, :NR],
                        lhsT=w1_g[:, k - w_st, f * P:(f + 1) * P],
                        rhs=rhs,
                        start=(k == 0),
                        stop=(k == KT1 - 1),
                    )

            pools.update(_mk_vec_tiles(nc, mybir, singles, io))
            pools["w3_sb"], pools["w4_sb"] = w3_sb, w4_sb

            # BN1 + ReLU fused: x1 = relu(c1 * s1 + t1), PSUM -> SBUF
            x1_sb = singles.tile([P, FT, NR], mm_dt)
            s1_sb, t1_sb = pools["s1_sb"], pools["t1_sb"]
            for f in range(FT):
                if f % 2 == 0:
                    nc.scalar.activation(
                        out=x1_sb[:, f, :], in_=c1[:, f, :NR], func=AF.Relu,
                        bias=t1_sb[:, f:f + 1], scale=s1_sb[:, f:f + 1],
                    )
                else:
                    nc.vector.tensor_scalar(
                        x1_sb[:, f, :], c1[:, f, :NR], s1_sb[:, f:f + 1],
                        t1_sb[:, f:f + 1], mybir.AluOpType.mult, mybir.AluOpType.add)
                    nc.vector.tensor_scalar_max(x1_sb[:, f, :], x1_sb[:, f, :], 0.0)

            pools["x2_sb"] = singles.tile([P, FT, NR], mm_dt, name="x2_sb")
            _emit_tail(nc, mybir, tc, pools, io, x1_sb,
                       lambda k, f: w2_sb[:, k, f * P:(f + 1) * P], mm_dt)

    nc.compile()
    return nc


def _build_program_ksplit():
    """conv1 contraction split across cores + ReduceScatter over rois."""
    from concourse import bacc
    import concourse.mybir as mybir
    import concourse.tile as tile

    f32 = mybir.dt.float32
    mm_dt = _mm_dt(mybir)
    AF = mybir.ActivationFunctionType

    nc = bacc.Bacc("TRN2", target_bir_lowering=False, debug=False,
                   num_devices=NCORES)
    io = _mk_io(nc, mybir, [KTC * P, NRT], [KTC * P, HID])

    with tile.TileContext(nc) as tc:
        with (
            tc.tile_pool(name="singles", bufs=1) as singles,
            tc.tile_pool(name="partial", bufs=2) as ppool,
            tc.tile_pool(name="psum", bufs=1, space="PSUM") as psum,
            tc.tile_pool(name="small", bufs=2) as small,
            tc.tile_pool(name="dram", bufs=1, space="DRAM") as dram,
        ):
            pools = {"singles": singles, "psum": psum, "small": small}
            pools.update(_mk_vec_tiles(nc, mybir, singles, io))

            w3_sb = singles.tile([P, FT, NCLS_P], mm_dt)
            nc.scalar.dma_start(w3_sb, io["w3"].ap().rearrange("(kt p) f -> p kt f", p=P))
            w4_sb = singles.tile([P, FT, NDEL_P], mm_dt)
            nc.sync.dma_start(w4_sb, io["w4"].ap().rearrange("(kt p) f -> p kt f", p=P))
            pools["w3_sb"], pools["w4_sb"] = w3_sb, w4_sb

            # resident per-core slices: 13 a-tiles [128, 2048] + 13 w1-tiles
            # [128, 1024], DMA'd in consumption order
            a_t3 = io["a_t"].ap().rearrange("(kt p) n -> p kt n", p=P)
            w1_3 = io["w1"].ap().rearrange("(kt p) f -> p kt f", p=P)
            a_sb, w1_sb = [], []
            for k in range(KTC):
                at = singles.tile([P, NRT], mm_dt, tag=f"ak{k}")
                nc.sync.dma_start(at, a_t3[:, k, :])
                a_sb.append(at)
                wt = singles.tile([P, HID], mm_dt, tag=f"wk{k}")
                nc.sync.dma_start(wt, w1_3[:, k, :])
                w1_sb.append(wt)

            in_bounce = dram.tile([NCORES, HID, NR], f32)
            out_bounce = dram.tile([HID, NR], f32)

            # conv1 partials: for each owner core rc, accumulate the local
            # K-slice's contribution to X1^T[:, rc*256:(rc+1)*256]
            for rc in range(NCORES):
                acc = psum.tile([P, FT, 512], f32, tag="acc")
                for k in range(KTC):
                    rhs = a_sb[k][:, rc * NR:(rc + 1) * NR]
                    for f in range(FT):
                        nc.tensor.matmul(
                            acc[:, f, :NR],
                            lhsT=w1_sb[k][:, f * P:(f + 1) * P],
                            rhs=rhs,
                            start=(k == 0),
                            stop=(k == KTC - 1),
                        )
                part = ppool.tile([P, FT, NR], f32, tag="part")
                nc.scalar.copy(part[:, 0:4, :], acc[:, 0:4, :NR])
                nc.vector.tensor_copy(part[:, 4:8, :], acc[:, 4:8, :NR])
                nc.sync.dma_start(
                    in_bounce[rc].rearrange("(kt p) n -> p kt n", p=P), part)

            nc.gpsimd.collective_compute(
                "ReduceScatter",
                mybir.AluOpType.add,
                replica_groups=[list(range(NCORES))],
                ins=[in_bounce.opt()],
                outs=[out_bounce.opt()],
            )

            # w2 arrives late, into the SBUF slots freed by the a-tiles
            w2_tiles = []
            for g in range(4):
                wt = singles.tile([P, 2, HID], mm_dt, tag=f"ak{3 + g}")
                nc.sync.dma_start(
                    wt, io["w2"].ap().rearrange("(kt p) f -> p kt f", p=P)
                    [:, 2 * g:2 * g + 2, :])
                w2_tiles.append(wt)

            x1_pre = singles.tile([P, FT, NR], f32, tag="ak0")
            nc.sync.dma_start(x1_pre, out_bounce.rearrange("(kt p) n -> p kt n", p=P))
            x1_sb = singles.tile([P, FT, NR], mm_dt, tag="ak1")
            s1_sb, t1_sb = pools["s1_sb"], pools["t1_sb"]
            for f in range(FT):
                nc.scalar.activation(
                    out=x1_sb[:, f, :], in_=x1_pre[:, f, :], func=AF.Relu,
                    bias=t1_sb[:, f:f + 1], scale=s1_sb[:, f:f + 1],
                )

            pools["x2_sb"] = singles.tile([P, FT, NR], mm_dt, tag="ak2", name="x2_sb")
            _emit_tail(nc, mybir, tc, pools, io, x1_sb,
                       lambda k, f: w2_tiles[k // 2][:, k % 2, f * P:(f + 1) * P],
                       mm_dt)

    nc.compile()
    return nc


def get_program():
    if "nc" not in _CACHE:
        _CACHE["nc"] = (_build_program_ksplit() if IMPL == "ksplit"
                        else _build_program_dp())
    return _CACHE["nc"]


def _round_f32r(x):
    """Round fp32 to the PE's FP32r (11-bit mantissa, TF32-like) format so the
    on-device rounding step is a no-op and accuracy matches round-to-nearest."""
    x = np.ascontiguousarray(x, np.float32)
    b = x.view(np.uint32).astype(np.uint64)
    return (((b + 0x800) & 0xFFFFF000).astype(np.uint32)).view(np.float32)


def _fold_bn(gamma, beta, mean, var, conv_b):
    s = np.asarray(gamma, np.float64) / np.sqrt(np.asarray(var, np.float64) + BN_EPS)
    t = (np.asarray(conv_b, np.float64) - np.asarray(mean, np.float64)) * s \
        + np.asarray(beta, np.float64)
    return s.astype(np.float32), t.astype(np.float32)


def prepare_in_maps(pooled_rois, conv1_w, conv1_b, bn1_gamma, bn1_beta, bn1_mean,
                    bn1_var, conv2_w, conv2_b, bn2_gamma, bn2_beta, bn2_mean,
                    bn2_var, logits_w, logits_b, delta_w, delta_b):
    f = np.float32
    a_all = np.asarray(pooled_rois, f).reshape(N_ROIS, K1).T  # [K1, N_ROIS]

    s1_np, t1_np = _fold_bn(bn1_gamma, bn1_beta, bn1_mean, bn1_var, conv1_b)
    s2_np, t2_np = _fold_bn(bn2_gamma, bn2_beta, bn2_mean, bn2_var, conv2_b)

    w3_np = np.zeros((HID, NCLS_P), f)
    w3_np[:, :NCLS] = np.asarray(logits_w, f)
    b3_np = np.zeros(NCLS_P, f)
    b3_np[:NCLS] = np.asarray(logits_b, f)
    w4_np = np.zeros((HID, NDEL_P), f)
    w4_np[:, :NDEL] = np.asarray(delta_w, f)
    b4_np = np.zeros(NDEL_P, f)
    b4_np[:NDEL] = np.asarray(delta_b, f)

    w1_np = np.ascontiguousarray(np.asarray(conv1_w, f).reshape(K1, HID))
    w2_np = np.ascontiguousarray(np.asarray(conv2_w, f))
    if USE_F32R:
        w1_np = _round_f32r(w1_np)
        w2_np = _round_f32r(w2_np)
        w3_np = _round_f32r(w3_np)
        w4_np = _round_f32r(w4_np)
        a_all = _round_f32r(a_all)
    elif MM_DTYPE == "bf16":
        import ml_dtypes
        bf16 = ml_dtypes.bfloat16
        w1_np = w1_np.astype(bf16)
        w2_np = w2_np.astype(bf16)
        w3_np = w3_np.astype(bf16)
        w4_np = w4_np.astype(bf16)
        a_all = a_all.astype(bf16)

    shared = {
        "w2": w2_np, "w3": w3_np, "w4": w4_np,
        "s1": s1_np, "t1": t1_np, "s2": s2_np, "t2": t2_np,
        "b3": b3_np, "b4": b4_np,
    }
    in_maps = []
    if IMPL == "ksplit":
        a_pad = np.zeros((K1P, NRT), a_all.dtype)
        a_pad[:K1, :N_ROIS] = a_all
        w1_pad = np.zeros((K1P, HID), w1_np.dtype)
        w1_pad[:K1] = w1_np
        kc = KTC * P
        for c in range(NCORES):
            in_maps.append({
                "a_t": np.ascontiguousarray(a_pad[c * kc:(c + 1) * kc]),
                "w1": np.ascontiguousarray(w1_pad[c * kc:(c + 1) * kc]),
                **shared,
            })
    else:
        for c in range(NCORES):
            a_c = np.zeros((K1, NR), a_all.dtype)
            a_c[:, :RPC] = a_all[:, c * RPC:(c + 1) * RPC]
            in_maps.append({"a_t": a_c, "w1": w1_np, **shared})
    return in_maps


def gather_outputs(results):
    if IMPL == "ksplit":
        # core c owns padded rois [256c, 256c+256); real rois stop at 2000
        def cat(key):
            parts = []
            for c, r in enumerate(results):
                lo = c * NR
                n = min(NR, max(0, N_ROIS - lo))
                if n:
                    parts.append(r[key][:n])
            return np.concatenate(parts, axis=0)
    else:
        def cat(key):
            return np.concatenate([r[key][:RPC] for r in results], axis=0)

    logits = cat("logits_out")
    probs = cat("probs_out")
    deltas = cat("deltas_out")
    return logits, probs, deltas.reshape(N_ROIS, NCLS, 4)


def kernel(**inputs):
    from concourse.bass_utils import run_bass_kernel_spmd

    nc = get_program()
    in_maps = prepare_in_maps(**inputs)
    trace = bool(os.environ.get("BBOX_TRACE"))
    kwargs = {}
    if trace:
        kwargs = {"trace": True, "tmpdir": os.environ.get("BBOX_TRACE_DIR") or None}
    res = run_bass_kernel_spmd(nc, in_maps, core_ids=list(range(NCORES)), **kwargs)
    if trace:
        print(f"HW exec time: {res.exec_time_ns} ns")
        if res.instructions_and_trace:
            print("trace path:", res.instructions_and_trace[1])
        _CACHE["last_results"] = res
    return gather_outputs(res.results)


# revision 41
# speedup vs baseline: 1.0558x; 1.0558x over previous
"""BBoxHead kernel for 8 Trainium2 NeuronCores.

Reference computation (per roi):
  x1 = relu(bn1(pooled_rois . conv1_w + b1))      # full 7x7x256 contraction -> 1024
  x2 = relu(bn2(x1 @ conv2_w + b2))               # 1024 -> 1024
  logits = x2 @ logits_w + logits_b               # 1024 -> 81
  probs  = softmax(logits)
  deltas = x2 @ delta_w + delta_b                 # 1024 -> 324 -> [81, 4]

Activations are kept feature-major on-chip (X^T layout, [features, rois]) so
every matmul consumes operands K-on-partitions with zero on-device transposes
of activations; the host pre-transposes the pooled rois once.  BN is folded
into a per-feature affine on the host and applied fused with ReLU in a single
ScalarE activation per tile (PSUM -> SBUF).

Two distribution strategies (BBOX_IMPL):
  dp      - data-parallel over rois (250/core); every core streams the full
            conv1_w.  Default.
  ksplit  - conv1's contraction split across cores + on-chip ReduceScatter.
            Kept for reference: the collective costs ~70-120us in this
            runtime, so it loses to dp despite moving half the bytes.

Matmul dtype (BBOX_MM_DTYPE) and measured results (8 cores, HW exec time of
the traced core; scale-relative absmax vs the fp32 jax reference):
  bf16 (default)  ~148-155us  err 4.9e-3   conv1 stream 25.7MB/core, HBM-bound
  f32r            ~260us      err 2.8e-4   TF32-like; weight loads serialize
  f32             ~431us      err ~1e-6    full fp32 (4 PE cycles/row)
"""

import os
import sys

sys.path.insert(0, "/opt/trn_rl_repo")
import numpy as np

N_ROIS = 2000
K1 = 12544          # 7*7*256 contraction for conv1
HID = 1024
NCLS = 81
NCLS_P = 128        # logits head padded to a full PE tile
NDEL = 324
NDEL_P = 384        # delta head padded to 3 full PE tiles
P = 128
KT1 = K1 // P       # 98 contraction tiles for conv1 (dp mode)
FT = HID // P       # 8 feature tiles
NCORES = 8
RPC = N_ROIS // NCORES  # 250 rois per core (dp mode)
NR = 256            # padded rois per core (f32r needs free dim >= 256)
BN_EPS = 1e-3

# Matmul operand dtype: "bf16" (1 cyc/row + fast weight load + half DMA),
# "f32r" (TF32-like, 1 cyc/row but serialized weight loads), "f32" (4 cyc/row).
MM_DTYPE = os.environ.get("BBOX_MM_DTYPE", "bf16")
USE_F32R = MM_DTYPE == "f32r"
IMPL = os.environ.get("BBOX_IMPL", "dp")

K1P = 13312          # conv1 contraction padded to 8*13 tiles of 128
KTC = 13             # conv1 k-tiles per core in ksplit mode
NRT = 2048           # padded total rois in ksplit mode (8 x 256)

_CACHE: dict = {}


def _mm_dt(mybir):
    return {"bf16": mybir.dt.bfloat16, "f32r": mybir.dt.float32r,
            "f32": mybir.dt.float32}[MM_DTYPE]


def _mk_io(nc, mybir, a_shape, w1_shape):
    f32 = mybir.dt.float32
    mm_dt = _mm_dt(mybir)
    io = {}
    io["a_t"] = nc.dram_tensor("a_t", a_shape, mm_dt, kind="ExternalInput")
    io["w1"] = nc.dram_tensor("w1", w1_shape, mm_dt, kind="ExternalInput")
    io["w2"] = nc.dram_tensor("w2", [HID, HID], mm_dt, kind="ExternalInput")
    io["w3"] = nc.dram_tensor("w3", [HID, NCLS_P], mm_dt, kind="ExternalInput")
    io["w4"] = nc.dram_tensor("w4", [HID, NDEL_P], mm_dt, kind="ExternalInput")
    for name, n in [("s1", HID), ("t1", HID), ("s2", HID), ("t2", HID),
                    ("b3", NCLS_P), ("b4", NDEL_P)]:
        io[name] = nc.dram_tensor(name, [n], f32, kind="ExternalInput")
    io["logits_out"] = nc.dram_tensor("logits_out", [NR, NCLS], f32, kind="ExternalOutput")
    io["probs_out"] = nc.dram_tensor("probs_out", [NR, NCLS], f32, kind="ExternalOutput")
    io["deltas_out"] = nc.dram_tensor("deltas_out", [NR, NDEL], f32, kind="ExternalOutput")
    return io


def _emit_tail(nc, mybir, tc, pools, io, x1_sb, w2_tiles, mm_dt):
    """conv2 + heads + softmax + transposed outputs, from feature-major x1."""
    from concourse.masks import make_identity

    f32 = mybir.dt.float32
    AF = mybir.ActivationFunctionType
    AX = mybir.AxisListType
    singles, psum, small = pools["singles"], pools["psum"], pools["small"]

    ident = singles.tile([P, P], f32)
    make_identity(nc, ident)

    s2_sb, t2_sb = pools["s2_sb"], pools["t2_sb"]
    b3_sb, b4_sb = pools["b3_sb"], pools["b4_sb"]
    w3_sb, w4_sb = pools["w3_sb"], pools["w4_sb"]

    # conv2: X2^T = W2^T @ X1^T
    c2 = psum.tile([P, FT, 512], f32, tag="acc")
    for f in range(FT):
        for k in range(FT):
            nc.tensor.matmul(
                c2[:, f, :NR],
                lhsT=w2_tiles(k, f),
                rhs=x1_sb[:, k, :],
                start=(k == 0),
                stop=(k == FT - 1),
            )

    x2_sb = pools["x2_sb"]
    for f in range(FT):
        nc.scalar.activation(
            out=x2_sb[:, f, :], in_=c2[:, f, :NR], func=AF.Relu,
            bias=t2_sb[:, f:f + 1], scale=s2_sb[:, f:f + 1],
        )

    # heads: logits^T into bank 0, deltas^T into banks 1..3
    c3 = psum.tile([P, FT, 512], f32, tag="acc")
    for k in range(FT):
        nc.tensor.matmul(
            c3[:, 0, :NR], lhsT=w3_sb[:, k, :], rhs=x2_sb[:, k, :],
            start=(k == 0), stop=(k == FT - 1),
        )
    for m in range(3):
        for k in range(FT):
            nc.tensor.matmul(
                c3[:, 1 + m, :NR],
                lhsT=w4_sb[:, k, m * P:(m + 1) * P],
                rhs=x2_sb[:, k, :],
                start=(k == 0), stop=(k == FT - 1),
            )

    # head bias adds on DVE so they overlap ScalarE's BN2 work
    l_sb = small.tile([P, NR], f32, tag="l")
    nc.vector.tensor_scalar_add(l_sb, c3[:, 0, :NR], b3_sb[:, 0:1])
    d_sb = small.tile([P, 3, NR], f32, tag="d")
    for m in range(3):
        nc.vector.tensor_scalar_add(d_sb[:, m, :], c3[:, 1 + m, :NR],
                                    b4_sb[:, m:m + 1])

    # transpose heads back to roi-major: 2 logit blocks + 6 delta blocks
    c4 = psum.tile([P, FT, 512], f32, tag="acc")
    for j in range(2):
        nc.tensor.transpose(c4[:, j, :P], l_sb[:, j * P:(j + 1) * P], ident)
    for m in range(3):
        for j in range(2):
            nc.tensor.transpose(c4[:, 2 + m * 2 + j, :P],
                                d_sb[:, m, j * P:(j + 1) * P], ident)

    lg_sb = small.tile([P, 2, NCLS], f32, tag="lg")
    pr_sb = small.tile([P, 2, NCLS], f32, tag="pr")
    dl_sb = small.tile([P, 2, NDEL], f32, tag="dl")
    for j in range(2):
        nc.vector.tensor_copy(lg_sb[:, j, :], c4[:, j, :NCLS])
        negmax = small.tile([P, 1], f32, tag="nm")
        nc.vector.reduce_max(negmax, c4[:, j, :NCLS], axis=AX.X, negate=True)
        esum = small.tile([P, 1], f32, tag="es")
        nc.scalar.activation(out=pr_sb[:, j, :], in_=c4[:, j, :NCLS],
                             func=AF.Exp, bias=negmax, scale=1.0,
                             accum_out=esum)
        rsum = small.tile([P, 1], f32, tag="rs")
        nc.vector.reciprocal(rsum, esum)
        nc.vector.tensor_scalar_mul(pr_sb[:, j, :], pr_sb[:, j, :], rsum)
        for m in range(3):
            mw = NDEL - m * P if m == 2 else P
            nc.vector.tensor_copy(dl_sb[:, j, m * P:m * P + mw],
                                  c4[:, 2 + m * 2 + j, :mw])

    nc.sync.dma_start(io["logits_out"].ap().rearrange("(j p) c -> p j c", p=P), lg_sb)
    nc.sync.dma_start(io["probs_out"].ap().rearrange("(j p) c -> p j c", p=P), pr_sb)
    nc.sync.dma_start(io["deltas_out"].ap().rearrange("(j p) c -> p j c", p=P), dl_sb)


def _mk_vec_tiles(nc, mybir, singles, io):
    f32 = mybir.dt.float32

    def vec_tile(name, cols):
        t = singles.tile([P, cols], f32, tag=name + "_sb")
        nc.sync.dma_start(t, io[name].ap().rearrange("(o p) -> p o", p=P))
        return t

    return {
        "s1_sb": vec_tile("s1", FT), "t1_sb": vec_tile("t1", FT),
        "s2_sb": vec_tile("s2", FT), "t2_sb": vec_tile("t2", FT),
        "b3_sb": vec_tile("b3", 1), "b4_sb": vec_tile("b4", 3),
    }


def _build_program_dp():
    """Data-parallel: 250 rois/core, full conv1_w streamed on every core."""
    from concourse import bacc
    import concourse.mybir as mybir
    import concourse.tile as tile

    f32 = mybir.dt.float32
    mm_dt = _mm_dt(mybir)
    AF = mybir.ActivationFunctionType

    nc = bacc.Bacc("TRN2", target_bir_lowering=False, debug=False,
                   num_devices=NCORES)
    io = _mk_io(nc, mybir, [K1, NR], [K1, HID])

    AG, KPG = 14, 7  # stream a_t in 14 groups of 7 k-tiles

    with tile.TileContext(nc) as tc:
        with (
            tc.tile_pool(name="singles", bufs=1) as singles,
            tc.tile_pool(name="astream", bufs=9 if MM_DTYPE == "bf16" else 4) as apool,
            tc.tile_pool(name="wstream", bufs=8 if MM_DTYPE == "bf16" else 3) as wpool,
            tc.tile_pool(name="psum", bufs=1, space="PSUM") as psum,
            tc.tile_pool(name="small", bufs=2) as small,
        ):
            pools = {"singles": singles, "psum": psum, "small": small}

            # conv1: accumulate X1^T = W1^T @ A^T over 98 k-tiles.
            # w1 streams on the sync HWDGE queue; the a-groups ride gpsimd so
            # the two streams don't head-of-line block each other, and all
            # tail-only loads are emitted after the loop.
            a_t3 = io["a_t"].ap().rearrange("(kt p) n -> p kt n", p=P)
            c1 = psum.tile([P, FT, 512], f32, tag="acc")

            def schedule(sizes, total):
                out, k = [], 0
                for s in sizes:
                    out.append((k, s))
                    k += s
                while k < total:
                    s = min(sizes[-1], total - k)
                    out.append((k, s))
                    k += s
                assert k == total
                return out

            AGS = schedule([4, 7], KT1)             # a-groups
            WGS = schedule([2, 2, 4], KT1)          # w1-groups
            a_map = {}
            for st, sz in AGS:
                a_map[st] = (st, sz)
            w_map = {}
            for st, sz in WGS:
                w_map[st] = (st, sz)

            w2_sb = w3_sb = w4_sb = None
            a_g = w1_g = None
            a_st = w_st = 0
            wq = 0
            for k in range(KT1):
                if k in a_map:
                    st, sz = a_map[k]
                    a_g = apool.tile([P, 7, NR], mm_dt, tag="a", name="a_g")
                    a_st = st
                    nc.scalar.dma_start(a_g[:, :sz, :], a_t3[:, st:st + sz, :])
                if k in w_map:
                    st, sz = w_map[k]
                    w1_g = wpool.tile([P, 4, HID], mm_dt, tag="w1", name="w1_g")
                    w_st = st
                    nc.sync.dma_start(
                        w1_g[:, :sz, :],
                        io["w1"].ap().rearrange("(kt p) f -> p kt f", p=P)[:, st:st + sz, :])
                if k == 76:
                    # tail weights are queued on the in-order HWDGE queues
                    # BEHIND the k=76 w1 groups: queue order defers them past
                    # the bandwidth-critical conv1 head (an idle gpsimd queue
                    # would transfer them immediately and steal early HBM bw)
                    w2_sb = singles.tile([P, FT, HID], mm_dt, name="w2_sb")
                    w2_3 = io["w2"].ap().rearrange("(kt p) f -> p kt f", p=P)
                    nc.scalar.dma_start(w2_sb[:, 0:4, :], w2_3[:, 0:4, :])
                    nc.scalar.dma_start(w2_sb[:, 4:8, :], w2_3[:, 4:8, :])
                if k == 84:
                    w3_sb = singles.tile([P, FT, NCLS_P], mm_dt, name="w3_sb")
                    nc.scalar.dma_start(w3_sb, io["w3"].ap().rearrange("(kt p) f -> p kt f", p=P))
                    w4_sb = singles.tile([P, FT, NDEL_P], mm_dt, name="w4_sb")
                    nc.scalar.dma_start(w4_sb, io["w4"].ap().rearrange("(kt p) f -> p kt f", p=P))
                rhs = a_g[:, k - a_st, :]
                for f in range(FT):
                    nc.tensor.matmul(
                        c1[:, f, :NR],
                        lhsT=w1_g[:, k - w_st, f * P:(f + 1) * P],
                        rhs=rhs,
                        start=(k == 0),
                        stop=(k == KT1 - 1),
                    )

            pools.update(_mk_vec_tiles(nc, mybir, singles, io))
            pools["w3_sb"], pools["w4_sb"] = w3_sb, w4_sb

            # BN1 + ReLU fused: x1 = relu(c1 * s1 + t1), PSUM -> SBUF
            x1_sb = singles.tile([P, FT, NR], mm_dt)
            s1_sb, t1_sb = pools["s1_sb"], pools["t1_sb"]
            for f in range(FT):
                nc.scalar.activation(
                    out=x1_sb[:, f, :], in_=c1[:, f, :NR], func=AF.Relu,
                    bias=t1_sb[:, f:f + 1], scale=s1_sb[:, f:f + 1],
                )

            pools["x2_sb"] = singles.tile([P, FT, NR], mm_dt, name="x2_sb")
            _emit_tail(nc, mybir, tc, pools, io, x1_sb,
                       lambda k, f: w2_sb[:, k, f * P:(f + 1) * P], mm_dt)

    nc.compile()
    return nc


def _build_program_ksplit():
    """conv1 contraction split across cores + ReduceScatter over rois."""
    from concourse import bacc
    import concourse.mybir as mybir
    import concourse.tile as tile

    f32 = mybir.dt.float32
    mm_dt = _mm_dt(mybir)
    AF = mybir.ActivationFunctionType

    nc = bacc.Bacc("TRN2", target_bir_lowering=False, debug=False,
                   num_devices=NCORES)
    io = _mk_io(nc, mybir, [KTC * P, NRT], [KTC * P, HID])

    with tile.TileContext(nc) as tc:
        with (
            tc.tile_pool(name="singles", bufs=1) as singles,
            tc.tile_pool(name="partial", bufs=2) as ppool,
            tc.tile_pool(name="psum", bufs=1, space="PSUM") as psum,
            tc.tile_pool(name="small", bufs=2) as small,
            tc.tile_pool(name="dram", bufs=1, space="DRAM") as dram,
        ):
            pools = {"singles": singles, "psum": psum, "small": small}
            pools.update(_mk_vec_tiles(nc, mybir, singles, io))

            w3_sb = singles.tile([P, FT, NCLS_P], mm_dt)
            nc.scalar.dma_start(w3_sb, io["w3"].ap().rearrange("(kt p) f -> p kt f", p=P))
            w4_sb = singles.tile([P, FT, NDEL_P], mm_dt)
            nc.sync.dma_start(w4_sb, io["w4"].ap().rearrange("(kt p) f -> p kt f", p=P))
            pools["w3_sb"], pools["w4_sb"] = w3_sb, w4_sb

            # resident per-core slices: 13 a-tiles [128, 2048] + 13 w1-tiles
            # [128, 1024], DMA'd in consumption order
            a_t3 = io["a_t"].ap().rearrange("(kt p) n -> p kt n", p=P)
            w1_3 = io["w1"].ap().rearrange("(kt p) f -> p kt f", p=P)
            a_sb, w1_sb = [], []
            for k in range(KTC):
                at = singles.tile([P, NRT], mm_dt, tag=f"ak{k}")
                nc.sync.dma_start(at, a_t3[:, k, :])
                a_sb.append(at)
                wt = singles.tile([P, HID], mm_dt, tag=f"wk{k}")
                nc.sync.dma_start(wt, w1_3[:, k, :])
                w1_sb.append(wt)

            in_bounce = dram.tile([NCORES, HID, NR], f32)
            out_bounce = dram.tile([HID, NR], f32)

            # conv1 partials: for each owner core rc, accumulate the local
            # K-slice's contribution to X1^T[:, rc*256:(rc+1)*256]
            for rc in range(NCORES):
                acc = psum.tile([P, FT, 512], f32, tag="acc")
                for k in range(KTC):
                    rhs = a_sb[k][:, rc * NR:(rc + 1) * NR]
                    for f in range(FT):
                        nc.tensor.matmul(
                            acc[:, f, :NR],
                            lhsT=w1_sb[k][:, f * P:(f + 1) * P],
                            rhs=rhs,
                            start=(k == 0),
                            stop=(k == KTC - 1),
                        )
                part = ppool.tile([P, FT, NR], f32, tag="part")
                nc.scalar.copy(part[:, 0:4, :], acc[:, 0:4, :NR])
                nc.vector.tensor_copy(part[:, 4:8, :], acc[:, 4:8, :NR])
                nc.sync.dma_start(
                    in_bounce[rc].rearrange("(kt p) n -> p kt n", p=P), part)

            nc.gpsimd.collective_compute(
                "ReduceScatter",
                mybir.AluOpType.add,
                replica_groups=[list(range(NCORES))],
                ins=[in_bounce.opt()],
                outs=[out_bounce.opt()],
            )

            # w2 arrives late, into the SBUF slots freed by the a-tiles
            w2_tiles = []
            for g in range(4):
                wt = singles.tile([P, 2, HID], mm_dt, tag=f"ak{3 + g}")
                nc.sync.dma_start(
                    wt, io["w2"].ap().rearrange("(kt p) f -> p kt f", p=P)
                    [:, 2 * g:2 * g + 2, :])
                w2_tiles.append(wt)

            x1_pre = singles.tile([P, FT, NR], f32, tag="ak0")
            nc.sync.dma_start(x1_pre, out_bounce.rearrange("(kt p) n -> p kt n", p=P))
            x1_sb = singles.tile([P, FT, NR], mm_dt, tag="ak1")
            s1_sb, t1_sb = pools["s1_sb"], pools["t1_sb"]
            for f in range(FT):
                nc.scalar.activation(
                    out=x1_sb[:, f, :], in_=x1_pre[:, f, :], func=AF.Relu,
                    bias=t1_sb[:, f:f + 1], scale=s1_sb[:, f:f + 1],
                )

            pools["x2_sb"] = singles.tile([P, FT, NR], mm_dt, tag="ak2", name="x2_sb")
            _emit_tail(nc, mybir, tc, pools, io, x1_sb,
                       lambda k, f: w2_tiles[k // 2][:, k % 2, f * P:(f + 1) * P],
                       mm_dt)

    nc.compile()
    return nc


def get_program():
    if "nc" not in _CACHE:
        _CACHE["nc"] = (_build_program_ksplit() if IMPL == "ksplit"
                        else _build_program_dp())
    return _CACHE["nc"]


def _round_f32r(x):
    """Round fp32 to the PE's FP32r (11-bit mantissa, TF32-like) format so the
    on-device rounding step is a no-op and accuracy matches round-to-nearest."""
    x = np.ascontiguousarray(x, np.float32)
    b = x.view(np.uint32).astype(np.uint64)
    return (((b + 0x800) & 0xFFFFF000).astype(np.uint32)).view(np.float32)


def _fold_bn(gamma, beta, mean, var, conv_b):
    s = np.asarray(gamma, np.float64) / np.sqrt(np.asarray(var, np.float64) + BN_EPS)
    t = (np.asarray(conv_b, np.float64) - np.asarray(mean, np.float64)) * s \
        + np.asarray(beta, np.float64)
    return s.astype(np.float32), t.astype(np.float32)


def prepare_in_maps(pooled_rois, conv1_w, conv1_b, bn1_gamma, bn1_beta, bn1_mean,
                    bn1_var, conv2_w, conv2_b, bn2_gamma, bn2_beta, bn2_mean,
                    bn2_var, logits_w, logits_b, delta_w, delta_b):
    f = np.float32
    a_all = np.asarray(pooled_rois, f).reshape(N_ROIS, K1).T  # [K1, N_ROIS]

    s1_np, t1_np = _fold_bn(bn1_gamma, bn1_beta, bn1_mean, bn1_var, conv1_b)
    s2_np, t2_np = _fold_bn(bn2_gamma, bn2_beta, bn2_mean, bn2_var, conv2_b)

    w3_np = np.zeros((HID, NCLS_P), f)
    w3_np[:, :NCLS] = np.asarray(logits_w, f)
    b3_np = np.zeros(NCLS_P, f)
    b3_np[:NCLS] = np.asarray(logits_b, f)
    w4_np = np.zeros((HID, NDEL_P), f)
    w4_np[:, :NDEL] = np.asarray(delta_w, f)
    b4_np = np.zeros(NDEL_P, f)
    b4_np[:NDEL] = np.asarray(delta_b, f)

    w1_np = np.ascontiguousarray(np.asarray(conv1_w, f).reshape(K1, HID))
    w2_np = np.ascontiguousarray(np.asarray(conv2_w, f))
    if USE_F32R:
        w1_np = _round_f32r(w1_np)
        w2_np = _round_f32r(w2_np)
        w3_np = _round_f32r(w3_np)
        w4_np = _round_f32r(w4_np)
        a_all = _round_f32r(a_all)
    elif MM_DTYPE == "bf16":
        import ml_dtypes
        bf16 = ml_dtypes.bfloat16
        w1_np = w1_np.astype(bf16)
        w2_np = w2_np.astype(bf16)
        w3_np = w3_np.astype(bf16)
        w4_np = w4_np.astype(bf16)
        a_all = a_all.astype(bf16)

    shared = {
        "w2": w2_np, "w3": w3_np, "w4": w4_np,
        "s1": s1_np, "t1": t1_np, "s2": s2_np, "t2": t2_np,
        "b3": b3_np, "b4": b4_np,
    }
    in_maps = []
    if IMPL == "ksplit":
        a_pad = np.zeros((K1P, NRT), a_all.dtype)
        a_pad[:K1, :N_ROIS] = a_all
        w1_pad = np.zeros((K1P, HID), w1_np.dtype)
        w1_pad[:K1] = w1_np
        kc = KTC * P
        for c in range(NCORES):
            in_maps.append({
                "a_t": np.ascontiguousarray(a_pad[c * kc:(c + 1) * kc]),
                "w1": np.ascontiguousarray(w1_pad[c * kc:(c + 1) * kc]),
                **shared,
            })
    else:
        for c in range(NCORES):
            a_c = np.zeros((K1, NR), a_all.dtype)
            a_c[:, :RPC] = a_all[:, c * RPC:(c + 1) * RPC]
            in_maps.append({"a_t": a_c, "w1": w1_np, **shared})
    return in_maps


def gather_outputs(results):
    if IMPL == "ksplit":
        # core c owns padded rois [256c, 256c+256); real rois stop at 2000
        def cat(key):
            parts = []
            for c, r in enumerate(results):
                lo = c * NR
                n = min(NR, max(0, N_ROIS - lo))
                if n:
                    parts.append(r[key][:n])
            return np.concatenate(parts, axis=0)
    else:
        def cat(key):
            return np.concatenate([r[key][:RPC] for r in results], axis=0)

    logits = cat("logits_out")
    probs = cat("probs_out")
    deltas = cat("deltas_out")
    return logits, probs, deltas.reshape(N_ROIS, NCLS, 4)


def kernel(**inputs):
    from concourse.bass_utils import run_bass_kernel_spmd

    nc = get_program()
    in_maps = prepare_in_maps(**inputs)
    trace = bool(os.environ.get("BBOX_TRACE"))
    kwargs = {}
    if trace:
        kwargs = {"trace": True, "tmpdir": os.environ.get("BBOX_TRACE_DIR") or None}
    res = run_bass_kernel_spmd(nc, in_maps, core_ids=list(range(NCORES)), **kwargs)
    if trace:
        print(f"HW exec time: {res.exec_time_ns} ns")
        if res.instructions_and_trace:
            print("trace path:", res.instructions_and_trace[1])
        _CACHE["last_results"] = res
    return gather_outputs(res.results)


# revision 42
# speedup vs baseline: 1.0784x; 1.0215x over previous
"""BBoxHead kernel for 8 Trainium2 NeuronCores.

Reference computation (per roi):
  x1 = relu(bn1(pooled_rois . conv1_w + b1))      # full 7x7x256 contraction -> 1024
  x2 = relu(bn2(x1 @ conv2_w + b2))               # 1024 -> 1024
  logits = x2 @ logits_w + logits_b               # 1024 -> 81
  probs  = softmax(logits)
  deltas = x2 @ delta_w + delta_b                 # 1024 -> 324 -> [81, 4]

Activations are kept feature-major on-chip (X^T layout, [features, rois]) so
every matmul consumes operands K-on-partitions with zero on-device transposes
of activations; the host pre-transposes the pooled rois once.  BN is folded
into a per-feature affine on the host and applied fused with ReLU in a single
ScalarE activation per tile (PSUM -> SBUF).

Two distribution strategies (BBOX_IMPL):
  dp      - data-parallel over rois (250/core); every core streams the full
            conv1_w.  Default.
  ksplit  - conv1's contraction split across cores + on-chip ReduceScatter.
            Kept for reference: the collective costs ~70-120us in this
            runtime, so it loses to dp despite moving half the bytes.

Matmul dtype (BBOX_MM_DTYPE) and measured results (8 cores, HW exec time of
the traced core; scale-relative absmax vs the fp32 jax reference):
  bf16 (default)  ~148-155us  err 4.9e-3   conv1 stream 25.7MB/core, HBM-bound
  f32r            ~260us      err 2.8e-4   TF32-like; weight loads serialize
  f32             ~431us      err ~1e-6    full fp32 (4 PE cycles/row)
"""

import os
import sys

sys.path.insert(0, "/opt/trn_rl_repo")
import numpy as np

N_ROIS = 2000
K1 = 12544          # 7*7*256 contraction for conv1
HID = 1024
NCLS = 81
NCLS_P = 128        # logits head padded to a full PE tile
NDEL = 324
NDEL_P = 384        # delta head padded to 3 full PE tiles
P = 128
KT1 = K1 // P       # 98 contraction tiles for conv1 (dp mode)
FT = HID // P       # 8 feature tiles
NCORES = 8
RPC = N_ROIS // NCORES  # 250 rois per core (dp mode)
NR = 256            # padded rois per core (f32r needs free dim >= 256)
BN_EPS = 1e-3

# Matmul operand dtype: "bf16" (1 cyc/row + fast weight load + half DMA),
# "f32r" (TF32-like, 1 cyc/row but serialized weight loads), "f32" (4 cyc/row).
MM_DTYPE = os.environ.get("BBOX_MM_DTYPE", "bf16")
USE_F32R = MM_DTYPE == "f32r"
IMPL = os.environ.get("BBOX_IMPL", "dp")

K1P = 13312          # conv1 contraction padded to 8*13 tiles of 128
KTC = 13             # conv1 k-tiles per core in ksplit mode
NRT = 2048           # padded total rois in ksplit mode (8 x 256)

_CACHE: dict = {}


def _mm_dt(mybir):
    return {"bf16": mybir.dt.bfloat16, "f32r": mybir.dt.float32r,
            "f32": mybir.dt.float32}[MM_DTYPE]


def _mk_io(nc, mybir, a_shape, w1_shape):
    f32 = mybir.dt.float32
    mm_dt = _mm_dt(mybir)
    io = {}
    io["a_t"] = nc.dram_tensor("a_t", a_shape, mm_dt, kind="ExternalInput")
    io["w1"] = nc.dram_tensor("w1", w1_shape, mm_dt, kind="ExternalInput")
    io["w2"] = nc.dram_tensor("w2", [HID, HID], mm_dt, kind="ExternalInput")
    io["w3"] = nc.dram_tensor("w3", [HID, NCLS_P], mm_dt, kind="ExternalInput")
    io["w4"] = nc.dram_tensor("w4", [HID, NDEL_P], mm_dt, kind="ExternalInput")
    for name, n in [("s1", HID), ("t1", HID), ("s2", HID), ("t2", HID),
                    ("b3", NCLS_P), ("b4", NDEL_P)]:
        io[name] = nc.dram_tensor(name, [n], f32, kind="ExternalInput")
    io["logits_out"] = nc.dram_tensor("logits_out", [NR, NCLS], f32, kind="ExternalOutput")
    io["probs_out"] = nc.dram_tensor("probs_out", [NR, NCLS], f32, kind="ExternalOutput")
    io["deltas_out"] = nc.dram_tensor("deltas_out", [NR, NDEL], f32, kind="ExternalOutput")
    return io


def _emit_tail(nc, mybir, tc, pools, io, x1_sb, w2_tiles, mm_dt):
    """conv2 + heads + softmax + transposed outputs, from feature-major x1."""
    from concourse.masks import make_identity

    f32 = mybir.dt.float32
    AF = mybir.ActivationFunctionType
    AX = mybir.AxisListType
    singles, psum, small = pools["singles"], pools["psum"], pools["small"]

    ident = singles.tile([P, P], f32)
    make_identity(nc, ident)

    s2_sb, t2_sb = pools["s2_sb"], pools["t2_sb"]
    b3_sb, b4_sb = pools["b3_sb"], pools["b4_sb"]
    w3_sb, w4_sb = pools["w3_sb"], pools["w4_sb"]

    # conv2: X2^T = W2^T @ X1^T
    c2 = psum.tile([P, FT, 512], f32, tag="acc")
    for f in range(FT):
        for k in range(FT):
            nc.tensor.matmul(
                c2[:, f, :NR],
                lhsT=w2_tiles(k, f),
                rhs=x1_sb[:, k, :],
                start=(k == 0),
                stop=(k == FT - 1),
            )

    x2_sb = pools["x2_sb"]
    for f in range(FT):
        nc.scalar.activation(
            out=x2_sb[:, f, :], in_=c2[:, f, :NR], func=AF.Relu,
            bias=t2_sb[:, f:f + 1], scale=s2_sb[:, f:f + 1],
        )

    # heads: logits^T into bank 0, deltas^T into banks 1..3
    c3 = psum.tile([P, FT, 512], f32, tag="acc")
    for k in range(FT):
        nc.tensor.matmul(
            c3[:, 0, :NR], lhsT=w3_sb[:, k, :], rhs=x2_sb[:, k, :],
            start=(k == 0), stop=(k == FT - 1),
        )
    for m in range(3):
        for k in range(FT):
            nc.tensor.matmul(
                c3[:, 1 + m, :NR],
                lhsT=w4_sb[:, k, m * P:(m + 1) * P],
                rhs=x2_sb[:, k, :],
                start=(k == 0), stop=(k == FT - 1),
            )

    # head bias adds on DVE so they overlap ScalarE's BN2 work
    l_sb = small.tile([P, NR], f32, tag="l")
    nc.vector.tensor_scalar_add(l_sb, c3[:, 0, :NR], b3_sb[:, 0:1])
    d_sb = small.tile([P, 3, NR], f32, tag="d")
    for m in range(3):
        nc.vector.tensor_scalar_add(d_sb[:, m, :], c3[:, 1 + m, :NR],
                                    b4_sb[:, m:m + 1])

    # transpose heads back to roi-major: 2 logit blocks + 6 delta blocks
    c4 = psum.tile([P, FT, 512], f32, tag="acc")
    for j in range(2):
        nc.tensor.transpose(c4[:, j, :P], l_sb[:, j * P:(j + 1) * P], ident)
    for m in range(3):
        for j in range(2):
            nc.tensor.transpose(c4[:, 2 + m * 2 + j, :P],
                                d_sb[:, m, j * P:(j + 1) * P], ident)

    lg_sb = small.tile([P, 2, NCLS], f32, tag="lg")
    pr_sb = small.tile([P, 2, NCLS], f32, tag="pr")
    dl_sb = small.tile([P, 2, NDEL], f32, tag="dl")
    for j in range(2):
        nc.vector.tensor_copy(lg_sb[:, j, :], c4[:, j, :NCLS])
        negmax = small.tile([P, 1], f32, tag="nm")
        nc.vector.reduce_max(negmax, c4[:, j, :NCLS], axis=AX.X, negate=True)
        esum = small.tile([P, 1], f32, tag="es")
        nc.scalar.activation(out=pr_sb[:, j, :], in_=c4[:, j, :NCLS],
                             func=AF.Exp, bias=negmax, scale=1.0,
                             accum_out=esum)
        rsum = small.tile([P, 1], f32, tag="rs")
        nc.vector.reciprocal(rsum, esum)
        nc.vector.tensor_scalar_mul(pr_sb[:, j, :], pr_sb[:, j, :], rsum)
        for m in range(3):
            mw = NDEL - m * P if m == 2 else P
            nc.vector.tensor_copy(dl_sb[:, j, m * P:m * P + mw],
                                  c4[:, 2 + m * 2 + j, :mw])

    # one output per queue so the final stores drain in parallel
    nc.sync.dma_start(io["logits_out"].ap().rearrange("(j p) c -> p j c", p=P), lg_sb)
    nc.scalar.dma_start(io["probs_out"].ap().rearrange("(j p) c -> p j c", p=P), pr_sb)
    nc.sync.dma_start(io["deltas_out"].ap().rearrange("(j p) c -> p j c", p=P), dl_sb)


def _mk_vec_tiles(nc, mybir, singles, io):
    f32 = mybir.dt.float32

    def vec_tile(name, cols):
        t = singles.tile([P, cols], f32, tag=name + "_sb")
        nc.sync.dma_start(t, io[name].ap().rearrange("(o p) -> p o", p=P))
        return t

    return {
        "s1_sb": vec_tile("s1", FT), "t1_sb": vec_tile("t1", FT),
        "s2_sb": vec_tile("s2", FT), "t2_sb": vec_tile("t2", FT),
        "b3_sb": vec_tile("b3", 1), "b4_sb": vec_tile("b4", 3),
    }


def _build_program_dp():
    """Data-parallel: 250 rois/core, full conv1_w streamed on every core."""
    from concourse import bacc
    import concourse.mybir as mybir
    import concourse.tile as tile

    f32 = mybir.dt.float32
    mm_dt = _mm_dt(mybir)
    AF = mybir.ActivationFunctionType

    nc = bacc.Bacc("TRN2", target_bir_lowering=False, debug=False,
                   num_devices=NCORES)
    io = _mk_io(nc, mybir, [K1, NR], [K1, HID])

    AG, KPG = 14, 7  # stream a_t in 14 groups of 7 k-tiles

    with tile.TileContext(nc) as tc:
        with (
            tc.tile_pool(name="singles", bufs=1) as singles,
            tc.tile_pool(name="astream", bufs=9 if MM_DTYPE == "bf16" else 4) as apool,
            tc.tile_pool(name="wstream", bufs=8 if MM_DTYPE == "bf16" else 3) as wpool,
            tc.tile_pool(name="psum", bufs=1, space="PSUM") as psum,
            tc.tile_pool(name="small", bufs=3) as small,
        ):
            pools = {"singles": singles, "psum": psum, "small": small}

            # conv1: accumulate X1^T = W1^T @ A^T over 98 k-tiles.
            # w1 streams on the sync HWDGE queue; the a-groups ride gpsimd so
            # the two streams don't head-of-line block each other, and all
            # tail-only loads are emitted after the loop.
            a_t3 = io["a_t"].ap().rearrange("(kt p) n -> p kt n", p=P)
            c1 = psum.tile([P, FT, 512], f32, tag="acc")

            def schedule(sizes, total):
                out, k = [], 0
                for s in sizes:
                    out.append((k, s))
                    k += s
                while k < total:
                    s = min(sizes[-1], total - k)
                    out.append((k, s))
                    k += s
                assert k == total
                return out

            AGS = schedule([4, 7], KT1)             # a-groups
            WGS = schedule([2, 2, 4], KT1)          # w1-groups
            a_map = {}
            for st, sz in AGS:
                a_map[st] = (st, sz)
            w_map = {}
            for st, sz in WGS:
                w_map[st] = (st, sz)

            w2_sb = w3_sb = w4_sb = None
            a_g = w1_g = None
            a_st = w_st = 0
            wq = 0
            for k in range(KT1):
                if k in a_map:
                    st, sz = a_map[k]
                    a_g = apool.tile([P, 7, NR], mm_dt, tag="a", name="a_g")
                    a_st = st
                    nc.scalar.dma_start(a_g[:, :sz, :], a_t3[:, st:st + sz, :])
                if k in w_map:
                    st, sz = w_map[k]
                    w1_g = wpool.tile([P, 4, HID], mm_dt, tag="w1", name="w1_g")
                    w_st = st
                    nc.sync.dma_start(
                        w1_g[:, :sz, :],
                        io["w1"].ap().rearrange("(kt p) f -> p kt f", p=P)[:, st:st + sz, :])
                if k == 76:
                    # tail weights are queued on the in-order HWDGE queues
                    # BEHIND the k=76 w1 groups: queue order defers them past
                    # the bandwidth-critical conv1 head (an idle gpsimd queue
                    # would transfer them immediately and steal early HBM bw)
                    w2_sb = singles.tile([P, FT, HID], mm_dt, name="w2_sb")
                    w2_3 = io["w2"].ap().rearrange("(kt p) f -> p kt f", p=P)
                    nc.scalar.dma_start(w2_sb[:, 0:4, :], w2_3[:, 0:4, :])
                    nc.scalar.dma_start(w2_sb[:, 4:8, :], w2_3[:, 4:8, :])
                if k == 84:
                    w3_sb = singles.tile([P, FT, NCLS_P], mm_dt, name="w3_sb")
                    nc.scalar.dma_start(w3_sb, io["w3"].ap().rearrange("(kt p) f -> p kt f", p=P))
                    w4_sb = singles.tile([P, FT, NDEL_P], mm_dt, name="w4_sb")
                    nc.scalar.dma_start(w4_sb, io["w4"].ap().rearrange("(kt p) f -> p kt f", p=P))
                rhs = a_g[:, k - a_st, :]
                for f in range(FT):
                    nc.tensor.matmul(
                        c1[:, f, :NR],
                        lhsT=w1_g[:, k - w_st, f * P:(f + 1) * P],
                        rhs=rhs,
                        start=(k == 0),
                        stop=(k == KT1 - 1),
                    )

            pools.update(_mk_vec_tiles(nc, mybir, singles, io))
            pools["w3_sb"], pools["w4_sb"] = w3_sb, w4_sb

            # BN1 + ReLU fused: x1 = relu(c1 * s1 + t1), PSUM -> SBUF
            x1_sb = singles.tile([P, FT, NR], mm_dt)
            s1_sb, t1_sb = pools["s1_sb"], pools["t1_sb"]
            for f in range(FT):
                nc.scalar.activation(
                    out=x1_sb[:, f, :], in_=c1[:, f, :NR], func=AF.Relu,
                    bias=t1_sb[:, f:f + 1], scale=s1_sb[:, f:f + 1],
                )

            pools["x2_sb"] = singles.tile([P, FT, NR], mm_dt, name="x2_sb")
            _emit_tail(nc, mybir, tc, pools, io, x1_sb,
                       lambda k, f: w2_sb[:, k, f * P:(f + 1) * P], mm_dt)

    nc.compile()
    return nc


def _build_program_ksplit():
    """conv1 contraction split across cores + ReduceScatter over rois."""
    from concourse import bacc
    import concourse.mybir as mybir
    import concourse.tile as tile

    f32 = mybir.dt.float32
    mm_dt = _mm_dt(mybir)
    AF = mybir.ActivationFunctionType

    nc = bacc.Bacc("TRN2", target_bir_lowering=False, debug=False,
                   num_devices=NCORES)
    io = _mk_io(nc, mybir, [KTC * P, NRT], [KTC * P, HID])

    with tile.TileContext(nc) as tc:
        with (
            tc.tile_pool(name="singles", bufs=1) as singles,
            tc.tile_pool(name="partial", bufs=2) as ppool,
            tc.tile_pool(name="psum", bufs=1, space="PSUM") as psum,
            tc.tile_pool(name="small", bufs=2) as small,
            tc.tile_pool(name="dram", bufs=1, space="DRAM") as dram,
        ):
            pools = {"singles": singles, "psum": psum, "small": small}
            pools.update(_mk_vec_tiles(nc, mybir, singles, io))

            w3_sb = singles.tile([P, FT, NCLS_P], mm_dt)
            nc.scalar.dma_start(w3_sb, io["w3"].ap().rearrange("(kt p) f -> p kt f", p=P))
            w4_sb = singles.tile([P, FT, NDEL_P], mm_dt)
            nc.sync.dma_start(w4_sb, io["w4"].ap().rearrange("(kt p) f -> p kt f", p=P))
            pools["w3_sb"], pools["w4_sb"] = w3_sb, w4_sb

            # resident per-core slices: 13 a-tiles [128, 2048] + 13 w1-tiles
            # [128, 1024], DMA'd in consumption order
            a_t3 = io["a_t"].ap().rearrange("(kt p) n -> p kt n", p=P)
            w1_3 = io["w1"].ap().rearrange("(kt p) f -> p kt f", p=P)
            a_sb, w1_sb = [], []
            for k in range(KTC):
                at = singles.tile([P, NRT], mm_dt, tag=f"ak{k}")
                nc.sync.dma_start(at, a_t3[:, k, :])
                a_sb.append(at)
                wt = singles.tile([P, HID], mm_dt, tag=f"wk{k}")
                nc.sync.dma_start(wt, w1_3[:, k, :])
                w1_sb.append(wt)

            in_bounce = dram.tile([NCORES, HID, NR], f32)
            out_bounce = dram.tile([HID, NR], f32)

            # conv1 partials: for each owner core rc, accumulate the local
            # K-slice's contribution to X1^T[:, rc*256:(rc+1)*256]
            for rc in range(NCORES):
                acc = psum.tile([P, FT, 512], f32, tag="acc")
                for k in range(KTC):
                    rhs = a_sb[k][:, rc * NR:(rc + 1) * NR]
                    for f in range(FT):
                        nc.tensor.matmul(
                            acc[:, f, :NR],
                            lhsT=w1_sb[k][:, f * P:(f + 1) * P],
                            rhs=rhs,
                            start=(k == 0),
                            stop=(k == KTC - 1),
                        )
                part = ppool.tile([P, FT, NR], f32, tag="part")
                nc.scalar.copy(part[:, 0:4, :], acc[:, 0:4, :NR])
                nc.vector.tensor_copy(part[:, 4:8, :], acc[:, 4:8, :NR])
                nc.sync.dma_start(
                    in_bounce[rc].rearrange("(kt p) n -> p kt n", p=P), part)

            nc.gpsimd.collective_compute(
                "ReduceScatter",
                mybir.AluOpType.add,
                replica_groups=[list(range(NCORES))],
                ins=[in_bounce.opt()],
                outs=[out_bounce.opt()],
            )

            # w2 arrives late, into the SBUF slots freed by the a-tiles
            w2_tiles = []
            for g in range(4):
                wt = singles.tile([P, 2, HID], mm_dt, tag=f"ak{3 + g}")
                nc.sync.dma_start(
                    wt, io["w2"].ap().rearrange("(kt p) f -> p kt f", p=P)
                    [:, 2 * g:2 * g + 2, :])
                w2_tiles.append(wt)

            x1_pre = singles.tile([P, FT, NR], f32, tag="ak0")
            nc.sync.dma_start(x1_pre, out_bounce.rearrange("(kt p) n -> p kt n", p=P))
            x1_sb = singles.tile([P, FT, NR], mm_dt, tag="ak1")
            s1_sb, t1_sb = pools["s1_sb"], pools["t1_sb"]
            for f in range(FT):
                nc.scalar.activation(
                    out=x1_sb[:, f, :], in_=x1_pre[:, f, :], func=AF.Relu,
                    bias=t1_sb[:, f:f + 1], scale=s1_sb[:, f:f + 1],
                )

            pools["x2_sb"] = singles.tile([P, FT, NR], mm_dt, tag="ak2", name="x2_sb")
            _emit_tail(nc, mybir, tc, pools, io, x1_sb,
                       lambda k, f: w2_tiles[k // 2][:, k % 2, f * P:(f + 1) * P],
                       mm_dt)

    nc.compile()
    return nc


def get_program():
    if "nc" not in _CACHE:
        _CACHE["nc"] = (_build_program_ksplit() if IMPL == "ksplit"
                        else _build_program_dp())
    return _CACHE["nc"]


def _round_f32r(x):
    """Round fp32 to the PE's FP32r (11-bit mantissa, TF32-like) format so the
    on-device rounding step is a no-op and accuracy matches round-to-nearest."""
    x = np.ascontiguousarray(x, np.float32)
    b = x.view(np.uint32).astype(np.uint64)
    return (((b + 0x800) & 0xFFFFF000).astype(np.uint32)).view(np.float32)


def _fold_bn(gamma, beta, mean, var, conv_b):
    s = np.asarray(gamma, np.float64) / np.sqrt(np.asarray(var, np.float64) + BN_EPS)
    t = (np.asarray(conv_b, np.float64) - np.asarray(mean, np.float64)) * s \
        + np.asarray(beta, np.float64)
    return s.astype(np.float32), t.astype(np.float32)


def prepare_in_maps(pooled_rois, conv1_w, conv1_b, bn1_gamma, bn1_beta, bn1_mean,
                    bn1_var, conv2_w, conv2_b, bn2_gamma, bn2_beta, bn2_mean,
                    bn2_var, logits_w, logits_b, delta_w, delta_b):
    f = np.float32
    a_all = np.asarray(pooled_rois, f).reshape(N_ROIS, K1).T  # [K1, N_ROIS]

    s1_np, t1_np = _fold_bn(bn1_gamma, bn1_beta, bn1_mean, bn1_var, conv1_b)
    s2_np, t2_np = _fold_bn(bn2_gamma, bn2_beta, bn2_mean, bn2_var, conv2_b)

    w3_np = np.zeros((HID, NCLS_P), f)
    w3_np[:, :NCLS] = np.asarray(logits_w, f)
    b3_np = np.zeros(NCLS_P, f)
    b3_np[:NCLS] = np.asarray(logits_b, f)
    w4_np = np.zeros((HID, NDEL_P), f)
    w4_np[:, :NDEL] = np.asarray(delta_w, f)
    b4_np = np.zeros(NDEL_P, f)
    b4_np[:NDEL] = np.asarray(delta_b, f)

    w1_np = np.ascontiguousarray(np.asarray(conv1_w, f).reshape(K1, HID))
    w2_np = np.ascontiguousarray(np.asarray(conv2_w, f))
    if USE_F32R:
        w1_np = _round_f32r(w1_np)
        w2_np = _round_f32r(w2_np)
        w3_np = _round_f32r(w3_np)
        w4_np = _round_f32r(w4_np)
        a_all = _round_f32r(a_all)
    elif MM_DTYPE == "bf16":
        import ml_dtypes
        bf16 = ml_dtypes.bfloat16
        w1_np = w1_np.astype(bf16)
        w2_np = w2_np.astype(bf16)
        w3_np = w3_np.astype(bf16)
        w4_np = w4_np.astype(bf16)
        a_all = a_all.astype(bf16)

    shared = {
        "w2": w2_np, "w3": w3_np, "w4": w4_np,
        "s1": s1_np, "t1": t1_np, "s2": s2_np, "t2": t2_np,
        "b3": b3_np, "b4": b4_np,
    }
    in_maps = []
    if IMPL == "ksplit":
        a_pad = np.zeros((K1P, NRT), a_all.dtype)
        a_pad[:K1, :N_ROIS] = a_all
        w1_pad = np.zeros((K1P, HID), w1_np.dtype)
        w1_pad[:K1] = w1_np
        kc = KTC * P
        for c in range(NCORES):
            in_maps.append({
                "a_t": np.ascontiguousarray(a_pad[c * kc:(c + 1) * kc]),
                "w1": np.ascontiguousarray(w1_pad[c * kc:(c + 1) * kc]),
                **shared,
            })
    else:
        for c in range(NCORES):
            a_c = np.zeros((K1, NR), a_all.dtype)
            a_c[:, :RPC] = a_all[:, c * RPC:(c + 1) * RPC]
            in_maps.append({"a_t": a_c, "w1": w1_np, **shared})
    return in_maps


def gather_outputs(results):
    if IMPL == "ksplit":
        # core c owns padded rois [256c, 256c+256); real rois stop at 2000
        def cat(key):
            parts = []
            for c, r in enumerate(results):
                lo = c * NR
                n = min(NR, max(0, N_ROIS - lo))
                if n:
                    parts.append(r[key][:n])
            return np.concatenate(parts, axis=0)
    else:
        def cat(key):
            return np.concatenate([r[key][:RPC] for r in results], axis=0)

    logits = cat("logits_out")
    probs = cat("probs_out")
    deltas = cat("deltas_out")
    return logits, probs, deltas.reshape(N_ROIS, NCLS, 4)


def kernel(**inputs):
    from concourse.bass_utils import run_bass_kernel_spmd

    nc = get_program()
    in_maps = prepare_in_maps(**inputs)
    trace = bool(os.environ.get("BBOX_TRACE"))
    kwargs = {}
    if trace:
        kwargs = {"trace": True, "tmpdir": os.environ.get("BBOX_TRACE_DIR") or None}
    res = run_bass_kernel_spmd(nc, in_maps, core_ids=list(range(NCORES)), **kwargs)
    if trace:
        print(f"HW exec time: {res.exec_time_ns} ns")
        if res.instructions_and_trace:
            print("trace path:", res.instructions_and_trace[1])
        _CACHE["last_results"] = res
    return gather_outputs(res.results)
